# revision 1
# baseline (speedup 1.0000x reference)
"""Delta-form spectral kernel for nn_Dynamics_2748779069592 (TRN2, 8 cores).

Identity: Out_n = Z0 + Qc[(g16^n - 1) .* W0]Qc^T + F_n,
          F_n = Qc[(DT*sum_{k<16n} g^k) .* Qtil]Qc^T
|g16^n - 1| <= 0.0203, so tf32's 4.9e-4 relative rounding contributes ~1e-5
of |Z| to the output. F_n and Qtil (O(0.26) magnitude) stay fp32.

Sharding: 8 cores as 2x4 grid — core c owns 8 batch elems (half c%2) and 4
output times (quarter c//2). No cross-core communication.

Inputs are host-preswizzled to the [128, 512] on-chip layout (partition p
holds grid rows p and p+128) so every input DMA is 128 contiguous 2KB reads.
Engine map: PE matmuls; ACT all PSUM->SBUF copies; DVE elementwise muls +
PSUM-add evacuations; GPSIMD the (F_j + z_e) SBUF adds.
"""
import sys

sys.path.insert(0, "/opt/trn_rl_repo")
import warnings

warnings.filterwarnings("ignore")
import numpy as np

N = 256
P = 128
NE = 8  # elems per core
NT = 4  # output times per core
NCORES = 8
DT = 1e-3
NU = 1e-2

_compiled = None


def tf32_round(x):
    u = np.asarray(x, dtype=np.float32).view(np.uint32).astype(np.uint64)
    r = ((u >> 13) + ((u >> 12) & 1)) << 13
    return (r & 0xFFFFFFFF).astype(np.uint32).view(np.float32)


def swz(x):
    """[..., 256, 256] -> [..., 128, 512] on-chip layout (rows p, p+128)."""
    sh = x.shape[:-2]
    return (
        x.reshape(sh + (2, P, N)).swapaxes(-3, -2).reshape(sh + (P, 2 * N))
    )


def _make_tables():
    C = np.zeros((N, N))
    i = np.arange(N)
    C[i, (i + 1) % N] = 1.0
    C[i, (i - 1) % N] = 1.0
    C[i, i] = -2.0
    lam, Qc = np.linalg.eigh(C)
    a = DT * NU
    g = 1.0 + a * (lam[:, None] + lam[None, :])
    an = np.empty((16, N, N))
    bn = np.empty((16, N, N))
    S = np.zeros_like(g)
    gk = np.ones_like(g)
    for k in range(16 * 16):
        S += gk
        gk *= g
        if (k + 1) % 16 == 0:
            t = (k + 1) // 16 - 1
            an[t] = gk - 1.0
            bn[t] = DT * S
    g16 = g**16
    # per time-quarter tq (n0 = 4*tq): A = g16^n0 - 1, B = g16^n0 * (g16 - 1)
    at = np.empty((4, N, N))
    bt = np.empty((4, N, N))
    for tq in range(4):
        gn0 = g16 ** (4 * tq)
        at[tq] = gn0 - 1.0
        bt[tq] = gn0 * (g16 - 1.0)
    return Qc, an, bn, at, bt


def _build():
    import concourse.bacc as bacc
    import concourse.mybir as mybir
    from concourse.tile import TileContext

    f32 = mybir.dt.float32
    f32r = mybir.dt.float32r
    nc = bacc.Bacc("TRN2", target_bir_lowering=False, debug=False)

    z_d = nc.dram_tensor("z", [NE, P, 2 * N], f32, kind="ExternalInput")
    zr_d = nc.dram_tensor("zr", [NE, P, 2 * N], f32r, kind="ExternalInput")
    q_d = nc.dram_tensor("q", [P, 2 * N], f32, kind="ExternalInput")
    qc_d = nc.dram_tensor("qc", [P, 2 * N], f32, kind="ExternalInput")
    qct_d = nc.dram_tensor("qct", [P, 2 * N], f32, kind="ExternalInput")
    qcr_d = nc.dram_tensor("qcr", [P, 2 * N], f32r, kind="ExternalInput")
    qctr_d = nc.dram_tensor("qctr", [P, 2 * N], f32r, kind="ExternalInput")
    at_d = nc.dram_tensor("at", [P, 2 * N], f32r, kind="ExternalInput")
    bt_d = nc.dram_tensor("bt", [P, 2 * N], f32r, kind="ExternalInput")
    bn_d = nc.dram_tensor("bn", [NT, P, 2 * N], f32, kind="ExternalInput")
    out_d = nc.dram_tensor("out", [NE, NT, P, 2 * N], f32, kind="ExternalOutput")

    with TileContext(nc) as tc:
        with (
            tc.tile_pool(name="const", bufs=1) as cpool,
            tc.tile_pool(name="zs", bufs=1) as zpool,
            tc.tile_pool(name="work", bufs=2) as wpool,
            tc.tile_pool(name="recycle", bufs=5) as rpool,
            tc.tile_pool(name="i1p", bufs=8) as i1pool,
            tc.tile_pool(name="ddp", bufs=6) as ddpool,
            tc.tile_pool(name="hp", bufs=9) as hpool,
            tc.tile_pool(name="vp", bufs=1) as vpool,
            tc.tile_pool(name="gp", bufs=12) as gpool,
            tc.tile_pool(name="jvp", bufs=8) as jvpool,
            tc.tile_pool(name="outp", bufs=8) as opool,
            tc.tile_pool(name="psum", bufs=8, space="PSUM") as psum,
        ):
            _uid = [0]

            def nm(tag):
                _uid[0] += 1
                return f"{tag}_{_uid[0]}"

            def loadc(pool, tag, dt_, dram_ap):
                t = pool.tile([P, 2 * N], dt_, tag=tag, name=nm(tag))
                nc.sync.dma_start(out=t[:, :], in_=dram_ap)
                return t

            # order matters for the head: Qtil chain needs q+qc, forwards need zr+qcr
            q_t = loadc(cpool, "q", f32, q_d.ap()[:, :])
            qc_t = loadc(cpool, "qc", f32, qc_d.ap()[:, :])
            qcr_t = loadc(cpool, "qcr", f32r, qcr_d.ap()[:, :])
            qctr_t = loadc(cpool, "qctr", f32r, qctr_d.ap()[:, :])
            qct_t = loadc(cpool, "qct", f32, qct_d.ap()[:, :])
            zr_t = [loadc(rpool, "zr", f32r, zr_d.ap()[e]) for e in range(NE)]
            at_t = loadc(cpool, "at", f32r, at_d.ap()[:, :])
            bt_t = loadc(cpool, "bt", f32r, bt_d.ap()[:, :])
            bn_t = [loadc(cpool, f"bn{j}", f32, bn_d.ap()[j]) for j in range(NT)]
            z_t = [loadc(rpool, "z", f32, z_d.ap()[e]) for e in range(NE)]

            def mm256(lhs_t, rhs_t, out_t, evac):
                """out = lhs.T @ rhs (256x256 mats in [128, 512] layout)."""
                for m in range(2):
                    pt = psum.tile([P, N], f32, tag="ps", name=nm("ps"))
                    for k in range(2):
                        nc.tensor.matmul(
                            pt[:, :],
                            lhs_t[:, N * k + P * m : N * k + P * m + P],
                            rhs_t[:, N * k : N * k + N],
                            start=(k == 0),
                            stop=(k == 1),
                        )
                    evac(out_t[:, N * m : N * m + N], pt[:, :])
                return out_t

            act_cp = lambda o, p: nc.scalar.copy(out=o, in_=p)
            dve_cp = lambda o, p: nc.vector.tensor_copy(o, p)

            # ---- f32r forward transforms: W0_e ----
            w0_t = []
            for e in range(NE):
                fm = mm256(zr_t[e], qcr_t, i1pool.tile([P, 2 * N], f32r, tag="i1", name=nm("fm")), act_cp)
                w0 = mm256(fm, qcr_t, rpool.tile([P, 2 * N], f32r, tag="w0", name=nm("w0")), act_cp)
                w0_t.append(w0)

            # ---- per elem e: U = Inv(A.*W0), V = Inv(B.*W0); h = z + U;
            #      then 4 outputs: Out_je = (j+1)*V + (F_j + h) ----
            jv_all = {}
            h_all = {}

            def uv_chain(e):
                dv = ddpool.tile([P, 2 * N], f32r, tag="dv", name=nm("dv"))
                dveng = nc.gpsimd if e >= 3 else nc.vector
                dveng.tensor_mul(dv[:, :], bt_t[:, :], w0_t[e][:, :])
                du = ddpool.tile([P, 2 * N], f32r, tag="du", name=nm("du"))
                nc.gpsimd.tensor_mul(du[:, :], at_t[:, :], w0_t[e][:, :])
                iu = mm256(du, qctr_t, i1pool.tile([P, 2 * N], f32r, tag="i1", name=nm("iu")), act_cp)
                # U stage-2: fold h = z + U into the PSUM evacuation (DVE)
                h_e = hpool.tile([P, 2 * N], f32, tag="h", name=nm("h"))
                for m in range(2):
                    pt = psum.tile([P, N], f32, tag="ps", name=nm("ps"))
                    for k in range(2):
                        nc.tensor.matmul(
                            pt[:, :],
                            iu[:, N * k + P * m : N * k + P * m + P],
                            qctr_t[:, N * k : N * k + N],
                            start=(k == 0),
                            stop=(k == 1),
                        )
                    nc.vector.tensor_add(
                        h_e[:, N * m : N * m + N], pt[:, :], z_t[e][:, N * m : N * m + N]
                    )
                iv = mm256(dv, qctr_t, i1pool.tile([P, 2 * N], f32r, tag="i1", name=nm("iv")), act_cp)
                v_e = vpool.tile([P, 2 * N], f32, tag=f"v{e}", name=nm("v"))
                mm256(iv, qctr_t, v_e, act_cp)
                jv_all[e] = v_e
                h_all[e] = h_e



            # ---- fp32 Qtil first (short PE prefix), F inverses after uv(0) ----
            m1q = mm256(q_t, qc_t, wpool.tile([P, 2 * N], f32, tag="m1q", name=nm("m1q")), act_cp)
            qtil = mm256(m1q, qc_t, wpool.tile([P, 2 * N], f32, tag="qtil", name=nm("qtil")), act_cp)

            for e in range(1):
                uv_chain(e)

            f_t = []
            for j in range(NT):
                rb = wpool.tile([P, 2 * N], f32, tag="rb", name=nm("rb"))
                nc.gpsimd.tensor_mul(rb[:, :], bn_t[j][:, :], qtil[:, :])
                f1 = mm256(rb, qct_t, wpool.tile([P, 2 * N], f32, tag="f1", name=nm("f1")), act_cp)
                f_t.append(mm256(f1, qct_t, cpool.tile([P, 2 * N], f32, tag=f"F{j}", name=nm("F")), act_cp))

            for e in range(1, NE):
                uv_chain(e)

            # ---- assembly phase (low priority; fills engine idle) ----
            for e in range(NE):
                v_e, h_e = jv_all[e], h_all[e]
                for j in range(NT):
                    g_t = gpool.tile([P, 2 * N], f32, tag="g", name=nm("g"))
                    nc.vector.tensor_add(g_t[:, :], f_t[j][:, :], h_e[:, :])
                    if j == 0:
                        src = v_e
                    else:
                        src = jvpool.tile([P, 2 * N], f32, tag="jv", name=nm("jv"))
                        nc.scalar.mul(src[:, :], v_e[:, :], float(j + 1))
                    o_t = opool.tile([P, 2 * N], f32, tag="o", name=nm("o"))
                    nc.vector.tensor_add(o_t[:, :], src[:, :], g_t[:, :])
                    nc.sync.dma_start(
                        out=out_d.ap()[e, j],
                        in_=o_t[:, :],
                    )

    nc.compile()
    return nc


def _get_compiled():
    global _compiled
    if _compiled is None:
        _compiled = _build()
    return _compiled


def _run(inputs_full, Q, trace=False):
    from concourse import bass_utils

    nc = _get_compiled()
    Qc, an, bn, at, bt = _make_tables()
    qc32 = Qc.astype(np.float32)
    qct32 = np.ascontiguousarray(Qc.T).astype(np.float32)
    z32 = np.ascontiguousarray(inputs_full.astype(np.float32))
    zs = swz(z32)
    zrs = tf32_round(zs)
    qs, qcs, qcts = swz(np.asarray(Q, np.float32)), swz(qc32), swz(qct32)
    bns = swz(bn)
    ats, bts = swz(at), swz(bt)
    in_maps = []
    for c in range(NCORES):
        eh = c % 2
        tq = c // 2
        in_maps.append(
            {
                "z": np.ascontiguousarray(zs[eh * NE : (eh + 1) * NE]),
                "zr": np.ascontiguousarray(zrs[eh * NE : (eh + 1) * NE]),
                "q": np.ascontiguousarray(qs),
                "qc": np.ascontiguousarray(qcs),
                "qct": np.ascontiguousarray(qcts),
                "qcr": tf32_round(qcs),
                "qctr": tf32_round(qcts),
                "at": tf32_round(np.ascontiguousarray(ats[tq])),
                "bt": tf32_round(np.ascontiguousarray(bts[tq])),
                "bn": np.ascontiguousarray(bns[tq * NT : (tq + 1) * NT]).astype(np.float32),
            }
        )
    kw = dict(trace=True) if trace else {}
    last_err = None
    for attempt in range(3):
        try:
            res = bass_utils.run_bass_kernel_spmd(
                nc, in_maps, core_ids=list(range(NCORES)), **kw
            )
            break
        except Exception as exc:  # rare transient device error; retry
            last_err = exc
            import time

            time.sleep(5)
    else:
        raise last_err
    out = np.empty((16, 16, N, N), dtype=np.float32)
    for c in range(NCORES):
        eh, tq = c % 2, c // 2
        r = res.results[c]["out"]  # [NE, NT, 128, 512] swizzled
        r = r.reshape(NE, NT, P, 2, N).swapaxes(2, 3).reshape(NE, NT, N, N)
        out[eh * NE : (eh + 1) * NE, tq * NT : (tq + 1) * NT] = r
    return out, res


def kernel(inputs, Q):
    inputs = np.ascontiguousarray(np.asarray(inputs, dtype=np.float32))
    Q = np.ascontiguousarray(np.asarray(Q, dtype=np.float32))
    out, _ = _run(inputs, Q, trace=False)
    return out



# revision 7
# speedup vs baseline: 1.0986x; 1.0986x over previous
"""Binomial-stencil kernel for nn_Dynamics_2748779069592 (TRN2, 8 cores).

Identity: the step is linear, Z_{k+1} = (I + a·L)Z_k + DT·Q with L the 2D
periodic Laplacian and a = DT*NU = 1e-5.  After m steps:
  Z_m = (I+aL)^m Z0 + DT·Σ_{k<m}(I+aL)^k Q
Binomial expansion (|a·λ(L)·m| <= 0.0205):
  (I+aL)^m     = I + m·a·L + C(m,2)a²L² + ...   (drop k>=2: 2.1e-4 rel)
  DT·Σ(I+aL)^k = DT[C(m,1) + C(m,2)aL + C(m,3)a²L²]Q + O(9e-8)
So  Out_{n} = Z + (m·a)·L(Z) + R_n,  R_n = DT[C(m,1)Q + C(m,2)a·LQ + C(m,3)a²·L²Q].

L(Z) = S_row@Z + colsum(Z): the row part is a banded circulant (diag -4)
done as 4 [128,128]x[128,256] PE matmuls in the swizzled layout; the col
part is shifted DVE adds, folded into the PSUM evacuation.
Per output: psum = (αj·I)@E_e + I@Z_e + I@R_j (f32r matmuls, 1 cyc/row),
evacuated by ACT or DVE to an fp16 tile (fp16 round = 4.9e-4) and DMA'd.

Sharding: 8 cores as 2x4 grid — core c owns 8 batch elems (half c%2) and 4
output times (quarter c//2). No cross-core communication.
"""
import sys

sys.path.insert(0, "/opt/trn_rl_repo")
import warnings

warnings.filterwarnings("ignore")
import numpy as np

N = 256
P = 128
NE = 8  # elems per core
NT = 4  # output times per core
NCORES = 8
DT = 1e-3
NU = 1e-2
A = DT * NU

_compiled = None


def swz(x):
    """[..., 256, 256] -> [..., 128, 512] on-chip layout (rows p, p+128)."""
    sh = x.shape[:-2]
    return (
        x.reshape(sh + (2, P, N)).swapaxes(-3, -2).reshape(sh + (P, 2 * N))
    )


def _make_tables():
    # Row-stencil S: S[r, r+-1 mod 256] = 1, S[r,r] = -4 (carries the -4).
    S = np.zeros((N, N), np.float32)
    i = np.arange(N)
    S[i, (i + 1) % N] = 1.0
    S[i, (i - 1) % N] = 1.0
    S[i, i] = -4.0
    st = np.empty((4, P, P), np.float32)  # [m*2+h] = S[m-block, h-block].T
    for m in range(2):
        for h in range(2):
            st[m * 2 + h] = S[P * m : P * m + P, P * h : P * h + P].T
    ident = np.eye(P, dtype=np.float32)
    return st, ident


def _core_scalars(tq):
    """Per-core (time-quarter) scalars: alphas and R-combination coefs."""

    def c2(m):
        return m * (m - 1) / 2.0

    def c3(m):
        return m * (m - 1) * (m - 2) / 6.0

    alph = []
    rc = []  # (c1, c2, c3) per j: R = c1*Q + c2*LQ + c3*L2Q
    for j in range(NT):
        m = 16 * (4 * tq + j + 1)
        alph.append(A * m)
        rc.append((DT * m, DT * c2(m) * A, DT * c3(m) * A * A))
    return alph, rc


def _build():
    import concourse.bacc as bacc
    import concourse.mybir as mybir
    from concourse.tile import TileContext

    f32r = mybir.dt.float32r
    f16 = mybir.dt.float16
    nc = bacc.Bacc("TRN2", target_bir_lowering=False, debug=False)

    z_d = nc.dram_tensor("z", [NE, P, 2 * N], f32r, kind="ExternalInput")
    q_d = nc.dram_tensor("q", [P, 2 * N], f32r, kind="ExternalInput")
    st_d = nc.dram_tensor("st", [4, P, P], f32r, kind="ExternalInput")
    id_d = nc.dram_tensor("ident", [P, P], f32r, kind="ExternalInput")
    # 16 per-core scalars: alpha_j (4) then rc flattened (12), replicated
    # across partitions so sc[:, k:k+1] is a per-partition scalar operand
    sc_d = nc.dram_tensor("sc", [P, 16], mybir.dt.float32, kind="ExternalInput")
    out_d = nc.dram_tensor("out", [NE, NT, P, 2 * N], f16, kind="ExternalOutput")

    with TileContext(nc) as tc:
        with (
            tc.tile_pool(name="const", bufs=1) as cpool,
            tc.tile_pool(name="zs", bufs=8) as zpool,
            tc.tile_pool(name="cs", bufs=3) as cspool,
            tc.tile_pool(name="es", bufs=3) as epool,
            tc.tile_pool(name="outp", bufs=8) as opool,
            tc.tile_pool(name="pse", bufs=2, space="PSUM") as pse,
            tc.tile_pool(name="pso", bufs=6, space="PSUM") as pso,
        ):
            _uid = [0]

            def nm(tag):
                _uid[0] += 1
                return f"{tag}_{_uid[0]}"

            # ---- const loads ----
            st_t = []
            for k in range(4):
                t = cpool.tile([P, P], f32r, tag=f"st{k}", name=nm("st"))
                nc.sync.dma_start(out=t[:, :], in_=st_d.ap()[k])
                st_t.append(t)
            id_t = cpool.tile([P, P], f32r, tag="id", name=nm("id"))
            nc.sync.dma_start(out=id_t[:, :], in_=id_d.ap()[:, :])
            sc_t = cpool.tile([P, 16], mybir.dt.float32, tag="sc", name=nm("sc"))
            nc.sync.dma_start(out=sc_t[:, :], in_=sc_d.ap()[:, :])
            q_t = cpool.tile([P, 2 * N], f32r, tag="q", name=nm("q"))
            nc.sync.dma_start(out=q_t[:, :], in_=q_d.ap()[:, :])
            z_t = []
            for e in range(NE):
                t = zpool.tile([P, 2 * N], f32r, tag=f"z{e}", name=nm("z"))
                nc.sync.dma_start(out=t[:, :], in_=z_d.ap()[e])
                z_t.append(t)

            # scaled identities built on GpSimd from id_t (alpha_j, rc_kj)
            aid_t = []
            rid_t = []
            for j in range(NT):
                t = cpool.tile([P, P], f32r, tag=f"aid{j}", name=nm("aid"))
                nc.gpsimd.tensor_scalar_mul(t[:, :], id_t[:, :], sc_t[:, j : j + 1])
                aid_t.append(t)
                row = []
                for k in range(3):
                    t = cpool.tile([P, P], f32r, tag=f"rid{j}_{k}", name=nm("rid"))
                    k_ = 4 + 3 * j + k
                    nc.gpsimd.tensor_scalar_mul(
                        t[:, :], id_t[:, :], sc_t[:, k_ : k_ + 1]
                    )
                    row.append(t)
                rid_t.append(row)

            def rowpart(ps, src_t, extra=None):
                """psum[128,512] = S_row @ src (+ extra accumulated)."""
                for m in range(2):
                    for h in range(2):
                        nc.tensor.matmul(
                            ps[:, N * m : N * m + N],
                            st_t[m * 2 + h][:, :],
                            src_t[:, N * h : N * h + N],
                            start=(h == 0),
                            stop=(h == 1 and extra is None),
                        )
                if extra is not None:
                    lhs, rhs = extra
                    nc.tensor.matmul(
                        ps[:, :], lhs[:, :], rhs[:, :], start=False, stop=True
                    )

            def colsum(cs, src_t, eng):
                """cs[:, c] = src[:, c-1] + src[:, c+1] per 256-col half, wrap."""
                for h in range(2):
                    b = N * h
                    eng.tensor_add(
                        cs[:, b + 1 : b + N - 1],
                        src_t[:, b : b + N - 2],
                        src_t[:, b + 2 : b + N],
                    )
                    eng.tensor_add(
                        cs[:, b : b + 1],
                        src_t[:, b + N - 1 : b + N],
                        src_t[:, b + 1 : b + 2],
                    )
                    eng.tensor_add(
                        cs[:, b + N - 1 : b + N],
                        src_t[:, b + N - 2 : b + N - 1],
                        src_t[:, b : b + 1],
                    )

            def lap(src_t, out_tag, eng):
                """Full Laplacian via PE rowpart + eng colsum, DVE evac."""
                cs = cspool.tile([P, 2 * N], f32r, tag="cs", name=nm("cs"))
                colsum(cs, src_t, eng)
                ps = pse.tile([P, 2 * N], mybir.dt.float32, tag="pse", name=nm("pse"))
                rowpart(ps, src_t)
                lt = epool.tile([P, 2 * N], f32r, tag=out_tag, name=nm(out_tag))
                nc.vector.tensor_add(lt[:, :], ps[:, :], cs[:, :])
                return lt

            # ---- forcing prep: LQ, L2Q, R_j ----
            lq_t = lap(q_t, "lq", nc.gpsimd)
            l2q_t = lap(lq_t, "l2q", nc.gpsimd)
            r_t = []
            for j in range(NT):
                ps = pse.tile([P, 2 * N], mybir.dt.float32, tag="pse", name=nm("psr"))
                nc.tensor.matmul(ps[:, :], rid_t[j][0][:, :], q_t[:, :], start=True, stop=False)
                nc.tensor.matmul(ps[:, :], rid_t[j][1][:, :], lq_t[:, :], start=False, stop=False)
                nc.tensor.matmul(ps[:, :], rid_t[j][2][:, :], l2q_t[:, :], start=False, stop=True)
                rt = cpool.tile([P, 2 * N], f32r, tag=f"r{j}", name=nm("r"))
                nc.scalar.copy(out=rt[:, :], in_=ps[:, :])
                r_t.append(rt)

            # ---- per elem: E = Lap(Z); 4 outputs ----
            for e in range(NE):
                eng = nc.gpsimd if e % 2 else nc.vector
                e_t = lap(z_t[e], "E", eng)
                for j in range(NT):
                    ps = pso.tile(
                        [P, 2 * N], mybir.dt.float32, tag="pso", name=nm("pso")
                    )
                    nc.tensor.matmul(ps[:, :], aid_t[j][:, :], e_t[:, :], start=True, stop=False)
                    nc.tensor.matmul(ps[:, :], id_t[:, :], z_t[e][:, :], start=False, stop=False)
                    nc.tensor.matmul(ps[:, :], id_t[:, :], r_t[j][:, :], start=False, stop=True)
                    o_t = opool.tile([P, 2 * N], f16, tag="o", name=nm("o"))
                    nc.scalar.copy(out=o_t[:, :], in_=ps[:, :])
                    nc.sync.dma_start(out=out_d.ap()[e, j], in_=o_t[:, :])

    nc.compile()
    return nc


def _get_compiled():
    global _compiled
    if _compiled is None:
        _compiled = _build()
    return _compiled


def _run(inputs_full, Q, trace=False):
    from concourse import bass_utils

    nc = _get_compiled()
    st, ident = _make_tables()
    z32 = np.ascontiguousarray(inputs_full.astype(np.float32))
    zs = swz(z32)
    qs = np.ascontiguousarray(swz(np.asarray(Q, np.float32)))
    in_maps = []
    for c in range(NCORES):
        eh = c % 2
        tq = c // 2
        alph, rc = _core_scalars(tq)
        sc = np.tile(
            np.array(alph + [v for row in rc for v in row], np.float32), (P, 1)
        )
        in_maps.append(
            {
                "z": np.ascontiguousarray(zs[eh * NE : (eh + 1) * NE]),
                "q": qs,
                "st": st,
                "ident": ident,
                "sc": sc,
            }
        )
    kw = dict(trace=True) if trace else {}
    last_err = None
    for attempt in range(3):
        try:
            res = bass_utils.run_bass_kernel_spmd(
                nc, in_maps, core_ids=list(range(NCORES)), **kw
            )
            break
        except Exception as exc:  # rare transient device error; retry
            last_err = exc
            import time

            time.sleep(5)
    else:
        raise last_err
    out = np.empty((16, 16, N, N), dtype=np.float32)
    for c in range(NCORES):
        eh, tq = c % 2, c // 2
        r = np.asarray(res.results[c]["out"], dtype=np.float32)
        r = r.reshape(NE, NT, P, 2, N).swapaxes(2, 3).reshape(NE, NT, N, N)
        out[eh * NE : (eh + 1) * NE, tq * NT : (tq + 1) * NT] = r
    return out, res


def kernel(inputs, Q):
    inputs = np.ascontiguousarray(np.asarray(inputs, dtype=np.float32))
    Q = np.ascontiguousarray(np.asarray(Q, dtype=np.float32))
    out, _ = _run(inputs, Q, trace=False)
    return out


# revision 8
# speedup vs baseline: 1.6278x; 1.4817x over previous
"""Binomial-stencil kernel for nn_Dynamics_2748779069592 (TRN2, 8 cores).

Identity: the step is linear, Z_{k+1} = (I + a·L)Z_k + DT·Q with L the 2D
periodic Laplacian and a = DT*NU = 1e-5.  After m steps:
  Z_m = (I+aL)^m Z0 + DT·Σ_{k<m}(I+aL)^k Q
Binomial expansion (|a·λ(L)·m| <= 0.0205):
  (I+aL)^m     = I + m·a·L + O(2.1e-4)
  DT·Σ(I+aL)^k = DT[C(m,1) + C(m,2)a·L]Q + O(1.8e-5)
So  Out_n = Z + (m·a)·L(Z) + R_n,  R_n = DT[C(m,1)Q + C(m,2)a·LQ].

L(Z) = S_row@Z + colsum(Z): row part as 4 [128,128]x[128,256] PE matmuls
(banded circulant, carries the -4 diag) in the swizzled layout; col part
as shifted GPSIMD adds, folded into the PSUM evacuation on DVE.
Per output, even elems: psum = (αj·I)@E + I@Z + I@R_j, ACT copy -> fp16;
odd elems: psum = (αj·I)@E + I@R_j, DVE add psum+Z -> fp16.  All scaled
identities are shipped from the host (per-core αj).

Sharding: 8 cores as 2x4 grid — core c owns 8 batch elems (half c%2) and 4
output times (quarter c//2). No cross-core communication.
"""
import sys

sys.path.insert(0, "/opt/trn_rl_repo")
import warnings

warnings.filterwarnings("ignore")
import numpy as np

N = 256
P = 128
NE = 8  # elems per core
NT = 4  # output times per core
NCORES = 8
DT = 1e-3
NU = 1e-2
A = DT * NU
NID = 5 + 2 * NT  # identity blocks: I, 4x alpha_j*I, 4x (c1j*I, c2j*I)

_compiled = None


def swz(x):
    """[..., 256, 256] -> [..., 128, 512] on-chip layout (rows p, p+128)."""
    sh = x.shape[:-2]
    return (
        x.reshape(sh + (2, P, N)).swapaxes(-3, -2).reshape(sh + (P, 2 * N))
    )


def _make_tables(tq):
    # Row-stencil S: S[r, r+-1 mod 256] = 1, S[r,r] = -4 (carries the -4).
    S = np.zeros((N, N), np.float32)
    i = np.arange(N)
    S[i, (i + 1) % N] = 1.0
    S[i, (i - 1) % N] = 1.0
    S[i, i] = -4.0
    st = np.empty((4, P, P), np.float32)  # [m*2+h] = S[m-block, h-block].T
    for m in range(2):
        for h in range(2):
            st[m * 2 + h] = S[P * m : P * m + P, P * h : P * h + P].T
    ident = np.eye(P, dtype=np.float32)
    ids = np.empty((NID, P, P), np.float32)
    ids[0] = ident
    for j in range(NT):
        m = 16 * (4 * tq + j + 1)
        ids[1 + j] = (A * m) * ident                       # alpha_j I
        ids[5 + 2 * j] = (DT * m) * ident                  # c1j I
        ids[6 + 2 * j] = (DT * A * m * (m - 1) / 2.0) * ident  # c2j I
    return st, ids


def _build():
    import concourse.bacc as bacc
    import concourse.mybir as mybir
    from concourse.tile import TileContext

    f32 = mybir.dt.float32
    f32r = mybir.dt.float32r
    f16 = mybir.dt.float16
    nc = bacc.Bacc("TRN2", target_bir_lowering=False, debug=False)

    z_d = nc.dram_tensor("z", [NE, P, 2 * N], f32r, kind="ExternalInput")
    q_d = nc.dram_tensor("q", [P, 2 * N], f32r, kind="ExternalInput")
    st_d = nc.dram_tensor("st", [4, P, P], f32r, kind="ExternalInput")
    id_d = nc.dram_tensor("ids", [NID, P, P], f32r, kind="ExternalInput")
    out_d = nc.dram_tensor("out", [NE, NT, P, 2 * N], f16, kind="ExternalOutput")

    with TileContext(nc) as tc:
        with (
            tc.tile_pool(name="const", bufs=1) as cpool,
            tc.tile_pool(name="zs", bufs=8) as zpool,
            tc.tile_pool(name="cs", bufs=3) as cspool,
            tc.tile_pool(name="es", bufs=3) as epool,
            tc.tile_pool(name="outp", bufs=8) as opool,
            tc.tile_pool(name="pse", bufs=2, space="PSUM") as pse,
            tc.tile_pool(name="pso", bufs=6, space="PSUM") as pso,
        ):
            _uid = [0]

            def nm(tag):
                _uid[0] += 1
                return f"{tag}_{_uid[0]}"

            # ---- const loads ----
            st_t = []
            for k in range(4):
                t = cpool.tile([P, P], f32r, tag=f"st{k}", name=nm("st"))
                nc.sync.dma_start(out=t[:, :], in_=st_d.ap()[k])
                st_t.append(t)
            id_t = []
            for k in range(NID):
                t = cpool.tile([P, P], f32r, tag=f"id{k}", name=nm("id"))
                nc.sync.dma_start(out=t[:, :], in_=id_d.ap()[k])
                id_t.append(t)
            q_t = cpool.tile([P, 2 * N], f32r, tag="q", name=nm("q"))
            nc.sync.dma_start(out=q_t[:, :], in_=q_d.ap()[:, :])
            z_t = []
            for e in range(NE):
                t = zpool.tile([P, 2 * N], f32r, tag=f"z{e}", name=nm("z"))
                nc.sync.dma_start(out=t[:, :], in_=z_d.ap()[e])
                z_t.append(t)

            def rowpart(ps, src_t):
                """psum[128,512] = S_row @ src."""
                for m in range(2):
                    for h in range(2):
                        nc.tensor.matmul(
                            ps[:, N * m : N * m + N],
                            st_t[m * 2 + h][:, :],
                            src_t[:, N * h : N * h + N],
                            start=(h == 0),
                            stop=(h == 1),
                        )

            def colsum(cs, src_t):
                """cs[:, c] = src[:, c-1] + src[:, c+1] per 256-col half, wrap."""
                for h in range(2):
                    b = N * h
                    nc.gpsimd.tensor_add(
                        cs[:, b + 1 : b + N - 1],
                        src_t[:, b : b + N - 2],
                        src_t[:, b + 2 : b + N],
                    )
                    nc.gpsimd.tensor_add(
                        cs[:, b : b + 1],
                        src_t[:, b + N - 1 : b + N],
                        src_t[:, b + 1 : b + 2],
                    )
                    nc.gpsimd.tensor_add(
                        cs[:, b + N - 1 : b + N],
                        src_t[:, b + N - 2 : b + N - 1],
                        src_t[:, b : b + 1],
                    )

            def lap(src_t, out_tag):
                """Full Laplacian via PE rowpart + GPSIMD colsum, DVE evac."""
                cs = cspool.tile([P, 2 * N], f32r, tag="cs", name=nm("cs"))
                colsum(cs, src_t)
                ps = pse.tile([P, 2 * N], f32, tag="pse", name=nm("pse"))
                rowpart(ps, src_t)
                lt = epool.tile([P, 2 * N], f32r, tag=out_tag, name=nm(out_tag))
                nc.vector.tensor_add(lt[:, :], ps[:, :], cs[:, :])
                return lt

            # ---- forcing prep: LQ, R_j = c1j Q + c2j LQ ----
            lq_t = lap(q_t, "lq")
            r_t = []
            for j in range(NT):
                ps = pse.tile([P, 2 * N], f32, tag="pse", name=nm("psr"))
                nc.tensor.matmul(ps[:, :], id_t[5 + 2 * j][:, :], q_t[:, :], start=True, stop=False)
                nc.tensor.matmul(ps[:, :], id_t[6 + 2 * j][:, :], lq_t[:, :], start=False, stop=True)
                rt = cpool.tile([P, 2 * N], f32r, tag=f"r{j}", name=nm("r"))
                nc.scalar.copy(out=rt[:, :], in_=ps[:, :])
                r_t.append(rt)

            # ---- per elem: E = Lap(Z); 4 outputs ----
            for e in range(NE):
                e_t = lap(z_t[e], "E")
                act_path = e % 2 == 0
                for j in range(NT):
                    ps = pso.tile([P, 2 * N], f32, tag="pso", name=nm("pso"))
                    nc.tensor.matmul(ps[:, :], id_t[1 + j][:, :], e_t[:, :], start=True, stop=False)
                    if act_path:
                        nc.tensor.matmul(ps[:, :], id_t[0][:, :], z_t[e][:, :], start=False, stop=False)
                    nc.tensor.matmul(ps[:, :], id_t[0][:, :], r_t[j][:, :], start=False, stop=True)
                    o_t = opool.tile([P, 2 * N], f16, tag="o", name=nm("o"))
                    if act_path:
                        nc.scalar.copy(out=o_t[:, :], in_=ps[:, :])
                    else:
                        nc.vector.tensor_add(o_t[:, :], ps[:, :], z_t[e][:, :])
                    nc.sync.dma_start(out=out_d.ap()[e, j], in_=o_t[:, :])

    nc.compile()
    return nc


def _get_compiled():
    global _compiled
    if _compiled is None:
        _compiled = _build()
    return _compiled


def _run(inputs_full, Q, trace=False):
    from concourse import bass_utils

    nc = _get_compiled()
    z32 = np.ascontiguousarray(inputs_full.astype(np.float32))
    zs = swz(z32)
    qs = np.ascontiguousarray(swz(np.asarray(Q, np.float32)))
    in_maps = []
    for c in range(NCORES):
        eh = c % 2
        tq = c // 2
        st, ids = _make_tables(tq)
        in_maps.append(
            {
                "z": np.ascontiguousarray(zs[eh * NE : (eh + 1) * NE]),
                "q": qs,
                "st": st,
                "ids": ids,
            }
        )
    kw = dict(trace=True) if trace else {}
    last_err = None
    for attempt in range(3):
        try:
            res = bass_utils.run_bass_kernel_spmd(
                nc, in_maps, core_ids=list(range(NCORES)), **kw
            )
            break
        except Exception as exc:  # rare transient device error; retry
            last_err = exc
            import time

            time.sleep(5)
    else:
        raise last_err
    out = np.empty((16, 16, N, N), dtype=np.float32)
    for c in range(NCORES):
        eh, tq = c % 2, c // 2
        r = np.asarray(res.results[c]["out"], dtype=np.float32)
        r = r.reshape(NE, NT, P, 2, N).swapaxes(2, 3).reshape(NE, NT, N, N)
        out[eh * NE : (eh + 1) * NE, tq * NT : (tq + 1) * NT] = r
    return out, res


def kernel(inputs, Q):
    inputs = np.ascontiguousarray(np.asarray(inputs, dtype=np.float32))
    Q = np.ascontiguousarray(np.asarray(Q, dtype=np.float32))
    out, _ = _run(inputs, Q, trace=False)
    return out


# revision 12
# speedup vs baseline: 1.7288x; 1.0620x over previous
"""Binomial-stencil kernel for nn_Dynamics_2748779069592 (TRN2, 8 cores).

Identity: the step is linear, Z_{k+1} = (I + a·L)Z_k + DT·Q with L the 2D
periodic Laplacian and a = DT*NU = 1e-5.  After m steps:
  Z_m = (I+aL)^m Z0 + DT·Σ_{k<m}(I+aL)^k Q
Binomial expansion (|a·λ(L)·m| <= 0.0205):
  (I+aL)^m     = I + m·a·L + O(2.1e-4)
  DT·Σ(I+aL)^k = DT[C(m,1) + C(m,2)a·L]Q + O(1.8e-5)
So  Out_n = Z + (m·a)·L(Z) + R_n,  R_n = DT[C(m,1)Q + C(m,2)a·LQ].

All SBUF data is fp16 (4.9e-4 rounding; PSUM stays f32).  L(Z) =
S_row@Z + colsum(Z): row part as 4 [128,128]x[128,256] PE matmuls (banded
circulant carrying the -4 diag); col part as 3 shifted 3D adds.  Outputs
are produced by three lanes to balance engines:
  A: psum = aj*I@E + I@Z + I@R_j (PE), ACT copy -> fp16
  B: psum = aj*I@E + I@R_j (PE), DVE add psum+Z -> fp16
  C: t = (E*aj)+Z (scalar_tensor_tensor), out = t+R_j  (DVE/Pool, no PSUM)
DMA is grouped: 5 input issues, 1 store per elem (4KB/partition contig).

Sharding: 8 cores as 2x4 grid — core c owns 8 batch elems (half c%2) and 4
output times (quarter c//2). No cross-core communication.
"""
import sys

sys.path.insert(0, "/opt/trn_rl_repo")
import warnings

warnings.filterwarnings("ignore")
import numpy as np

N = 256
P = 128
NE = 8  # elems per core
NT = 4  # output times per core
NCORES = 8
DT = 1e-3
NU = 1e-2
A = DT * NU
NID = 13  # I, 4x aj*I, 4x (c1j*I, c2j*I)

# per-(e,j) output lane: 'A' (PE3+ACT), 'B' (PE2+DVE), 'C' (DVE stt + Pool add)
LANES = {}
for _e in range(NE):
    for _j in range(NT):
        LANES[(_e, _j)] = ["A", "C", "B", "C"][_j] if _e % 2 == 0 else ["C", "A", "A", "C"][_j]
# C-lane second-op engine: all Pool (stt is DVE-only on TRN2)
C_ADD_DVE = set()
# E-evac engine per lap source: 'D' = DVE add, 'A' = PE I@cs + ACT copy
EVAC = {"q": "A", 0: "D", 1: "A", 2: "A", 3: "D", 4: "D", 5: "A", 6: "A", 7: "D"}
# colsum engine per lap source: True = Pool, False = DVE
CS_POOL = {"q": True, 0: True, 1: False, 2: True, 3: False, 4: True, 5: False, 6: True, 7: False}

_compiled = None


def _make_tables(tq):
    # Row-stencil S: S[r, r+-1 mod 256] = 1, S[r,r] = -4 (carries the -4).
    S = np.zeros((N, N), np.float32)
    i = np.arange(N)
    S[i, (i + 1) % N] = 1.0
    S[i, (i - 1) % N] = 1.0
    S[i, i] = -4.0
    st = np.empty((P, 4 * P), np.float16)  # [m*2+h] = S[m-block, h-block].T
    for m in range(2):
        for h in range(2):
            st[:, P * (m * 2 + h) : P * (m * 2 + h + 1)] = S[
                P * m : P * m + P, P * h : P * h + P
            ].T
    ident = np.eye(P, dtype=np.float32)
    ids = np.empty((P, NID * P), np.float16)
    sc = np.zeros(16, np.float32)
    ids[:, 0:P] = ident
    for j in range(NT):
        m = 16 * (4 * tq + j + 1)
        ids[:, (1 + j) * P : (2 + j) * P] = (A * m) * ident
        ids[:, (5 + 2 * j) * P : (6 + 2 * j) * P] = (DT * m) * ident
        ids[:, (6 + 2 * j) * P : (7 + 2 * j) * P] = (
            DT * A * m * (m - 1) / 2.0
        ) * ident
        sc[j] = A * m
    return st, ids, np.tile(sc, (P, 1))


def _build():
    import concourse.bacc as bacc
    import concourse.mybir as mybir
    from concourse.tile import TileContext

    f32 = mybir.dt.float32
    f16 = mybir.dt.float16
    nc = bacc.Bacc("TRN2", target_bir_lowering=False, debug=False)

    # host layouts put partition dim first so grouped DMAs are contiguous
    z_d = nc.dram_tensor("z", [P, NE, 2, N], f16, kind="ExternalInput")
    q_d = nc.dram_tensor("q", [P, 2, N], f16, kind="ExternalInput")
    st_d = nc.dram_tensor("st", [P, 4 * P], f16, kind="ExternalInput")
    id_d = nc.dram_tensor("ids", [P, NID * P], f16, kind="ExternalInput")
    sc_d = nc.dram_tensor("sc", [P, 16], f32, kind="ExternalInput")
    out_d = nc.dram_tensor("out", [P, NE, NT, 2, N], f16, kind="ExternalOutput")

    with TileContext(nc) as tc:
        with (
            tc.tile_pool(name="const", bufs=1) as cpool,
            tc.tile_pool(name="zs", bufs=1) as zpool,
            tc.tile_pool(name="cs", bufs=3) as cspool,
            tc.tile_pool(name="es", bufs=4) as epool,
            tc.tile_pool(name="ct", bufs=4) as ctpool,
            tc.tile_pool(name="outp", bufs=3) as opool,
            tc.tile_pool(name="pse", bufs=2, space="PSUM") as pse,
            tc.tile_pool(name="pso", bufs=6, space="PSUM") as pso,
        ):
            _uid = [0]

            def nm(tag):
                _uid[0] += 1
                return f"{tag}_{_uid[0]}"

            # ---- const loads (grouped) ----
            st_t = cpool.tile([P, 4 * P], f16, tag="st", name=nm("st"))
            nc.sync.dma_start(out=st_t[:, :], in_=st_d.ap()[:, :])
            id_t = cpool.tile([P, NID * P], f16, tag="ids", name=nm("ids"))
            nc.sync.dma_start(out=id_t[:, :], in_=id_d.ap()[:, :])
            sc_t = cpool.tile([P, 16], f32, tag="sc", name=nm("sc"))
            nc.sync.dma_start(out=sc_t[:, :], in_=sc_d.ap()[:, :])
            q_t = cpool.tile([P, 2, N], f16, tag="q", name=nm("q"))
            nc.sync.dma_start(out=q_t[:, :, :], in_=q_d.ap()[:, :, :])
            zA = zpool.tile([P, 4, 2, N], f16, tag="zA", name=nm("zA"))
            nc.sync.dma_start(out=zA[:, :, :, :], in_=z_d.ap()[:, 0:4])
            zB = zpool.tile([P, 4, 2, N], f16, tag="zB", name=nm("zB"))
            nc.sync.dma_start(out=zB[:, :, :, :], in_=z_d.ap()[:, 4:8])

            def zt(e):
                return (zA if e < 4 else zB)[:, e % 4]

            def idb(k):  # identity block k as [128,128] lhsT
                return id_t[:, k * P : (k + 1) * P]

            def stb(m, h):
                return st_t[:, (m * 2 + h) * P : (m * 2 + h + 1) * P]

            def rowpart(ps, src3, stop_after=True):
                """psum[:, m, :] = S_row @ src (3D [P,2,N] views)."""
                for m in range(2):
                    for h in range(2):
                        nc.tensor.matmul(
                            ps[:, m, :],
                            stb(m, h),
                            src3[:, h, :],
                            start=(h == 0),
                            stop=(h == 1 and stop_after),
                        )

            def colsum(cs, src3, pool):
                """cs[:,:,c] = src[:,:,c-1] + src[:,:,c+1], periodic (3 ops)."""
                eng = nc.gpsimd if pool else nc.vector
                eng.tensor_add(
                    cs[:, :, 1 : N - 1], src3[:, :, 0 : N - 2], src3[:, :, 2:N]
                )
                eng.tensor_add(
                    cs[:, :, 0:1], src3[:, :, N - 1 : N], src3[:, :, 1:2]
                )
                eng.tensor_add(
                    cs[:, :, N - 1 : N], src3[:, :, N - 2 : N - 1], src3[:, :, 0:1]
                )
            def lap(src3, key, out_tag):
                """E = S_row@src + colsum(src) -> fp16 tile [P,2,N]."""
                cs = cspool.tile([P, 2, N], f16, tag="cs", name=nm("cs"))
                colsum(cs, src3, CS_POOL[key])
                ps = pse.tile([P, 2, N], f32, tag="pse", name=nm("pse"))
                via_act = EVAC[key] == "A"
                rowpart(ps, src3, stop_after=not via_act)
                lt = epool.tile([P, 2, N], f16, tag=out_tag, name=nm(out_tag))
                if via_act:
                    nc.tensor.matmul(
                        ps[:, :, :], idb(0), cs[:, :, :], start=False, stop=True
                    )
                    nc.scalar.copy(out=lt[:, :, :], in_=ps[:, :, :])
                else:
                    nc.vector.tensor_add(lt[:, :, :], ps[:, :, :], cs[:, :, :])
                return lt

            # ---- forcing prep: LQ, R_j = c1j Q + c2j LQ ----
            lq_t = lap(q_t, "q", "lq")
            r_t = []
            for j in range(NT):
                ps = pse.tile([P, 2, N], f32, tag="pse", name=nm("psr"))
                nc.tensor.matmul(ps[:, :, :], idb(5 + 2 * j), q_t[:, :, :], start=True, stop=False)
                nc.tensor.matmul(ps[:, :, :], idb(6 + 2 * j), lq_t[:, :, :], start=False, stop=True)
                rt = cpool.tile([P, 2, N], f16, tag=f"r{j}", name=nm("r"))
                nc.scalar.copy(out=rt[:, :, :], in_=ps[:, :, :])
                r_t.append(rt)

            # ---- per elem: E = Lap(Z); 4 outputs via lanes ----
            for e in range(NE):
                z3 = zt(e)
                e_t = lap(z3, e, "E")
                o_t = opool.tile([P, NT, 2, N], f16, tag="o", name=nm("o"))
                for j in range(NT):
                    lane = LANES[(e, j)]
                    oj = o_t[:, j]
                    if lane in ("A", "B"):
                        ps = pso.tile([P, 2, N], f32, tag="pso", name=nm("pso"))
                        nc.tensor.matmul(ps[:, :, :], idb(1 + j), e_t[:, :, :], start=True, stop=False)
                        if lane == "A":
                            nc.tensor.matmul(ps[:, :, :], idb(0), z3[:, :, :], start=False, stop=False)
                        nc.tensor.matmul(ps[:, :, :], idb(0), r_t[j][:, :, :], start=False, stop=True)
                        if lane == "A":
                            nc.scalar.copy(out=oj[:, :, :], in_=ps[:, :, :])
                        else:
                            nc.vector.tensor_add(oj[:, :, :], ps[:, :, :], z3[:, :, :])
                    else:  # C: no PSUM
                        import concourse.mybir as mybir

                        t1 = ctpool.tile([P, 2, N], f16, tag="ct", name=nm("ct"))
                        nc.vector.scalar_tensor_tensor(
                            t1[:, :, :],
                            e_t[:, :, :],
                            sc_t[:, j : j + 1],
                            z3[:, :, :],
                            mybir.AluOpType.mult,
                            mybir.AluOpType.add,
                        )
                        nc.gpsimd.tensor_add(
                            oj[:, :, :], t1[:, :, :], r_t[j][:, :, :]
                        )
                nc.sync.dma_start(out=out_d.ap()[:, e], in_=o_t[:, :, :, :])

    nc.compile()
    return nc


def _get_compiled():
    global _compiled
    if _compiled is None:
        _compiled = _build()
    return _compiled


def _run(inputs_full, Q, trace=False):
    from concourse import bass_utils

    nc = _get_compiled()
    z32 = np.asarray(inputs_full, np.float32)
    # [B,256,256] -> per-core [P, NE, 2, N] fp16 (partition-major)
    zh = z32.reshape(16, 2, P, N).astype(np.float16)  # [B, half, p, col]
    qh = np.ascontiguousarray(
        np.asarray(Q, np.float32).reshape(2, P, N).transpose(1, 0, 2)
    ).astype(np.float16)  # [P,2,N]
    in_maps = []
    for c in range(NCORES):
        eh = c % 2
        tq = c // 2
        st, ids, sc = _make_tables(tq)
        zc = zh[eh * NE : (eh + 1) * NE]  # [NE,2,P,N]
        in_maps.append(
            {
                "z": np.ascontiguousarray(zc.transpose(2, 0, 1, 3)),
                "q": qh,
                "st": st,
                "ids": ids,
                "sc": sc,
            }
        )
    kw = dict(trace=True) if trace else {}
    last_err = None
    for attempt in range(3):
        try:
            res = bass_utils.run_bass_kernel_spmd(
                nc, in_maps, core_ids=list(range(NCORES)), **kw
            )
            break
        except Exception as exc:  # rare transient device error; retry
            last_err = exc
            import time

            time.sleep(5)
    else:
        raise last_err
    out = np.empty((16, 16, N, N), dtype=np.float32)
    for c in range(NCORES):
        eh, tq = c % 2, c // 2
        r = np.asarray(res.results[c]["out"], dtype=np.float32)
        # [P, NE, NT, 2, N] -> [NE, NT, 2, P, N] -> [NE, NT, 256, 256]
        r = r.transpose(1, 2, 3, 0, 4).reshape(NE, NT, N, N)
        out[eh * NE : (eh + 1) * NE, tq * NT : (tq + 1) * NT] = r
    return out, res


def kernel(inputs, Q):
    inputs = np.ascontiguousarray(np.asarray(inputs, dtype=np.float32))
    Q = np.ascontiguousarray(np.asarray(Q, dtype=np.float32))
    out, _ = _run(inputs, Q, trace=False)
    return out


# revision 14
# speedup vs baseline: 1.8014x; 1.0420x over previous
"""Binomial-stencil kernel for nn_Dynamics_2748779069592 (TRN2, 8 cores).

Identity: the step is linear, Z_{k+1} = (I + a·L)Z_k + DT·Q with L the 2D
periodic Laplacian and a = DT*NU = 1e-5.  After m steps:
  Z_m = (I+aL)^m Z0 + DT·Σ_{k<m}(I+aL)^k Q
Binomial expansion (|a·λ(L)·m| <= 0.0205):
  (I+aL)^m     = I + m·a·L + O(2.1e-4)
  DT·Σ(I+aL)^k = DT[C(m,1) + C(m,2)a·L]Q + O(1.8e-5)
So  Out_n = Z + (m·a)·L(Z) + R_n,  R_n = DT[C(m,1)Q + C(m,2)a·LQ].

All SBUF data is fp16 (4.9e-4 rounding; PSUM stays f32).  L(Z) =
S_row@Z + colsum(Z): row part as 4 [128,128]x[128,256] PE matmuls (banded
circulant carrying the -4 diag); col part as 3 shifted 3D adds.  Outputs
are produced by three lanes to balance engines:
  A: psum = aj*I@E + I@Z + I@R_j (PE), ACT copy -> fp16
  B: psum = aj*I@E + I@R_j (PE), DVE add psum+Z -> fp16
  C: t = (E*aj)+Z (scalar_tensor_tensor), out = t+R_j  (DVE/Pool, no PSUM)
DMA is grouped: 5 input issues, 1 store per elem (4KB/partition contig).

Sharding: 8 cores as 2x4 grid — core c owns 8 batch elems (half c%2) and 4
output times (quarter c//2). No cross-core communication.
"""
import sys

sys.path.insert(0, "/opt/trn_rl_repo")
import warnings

warnings.filterwarnings("ignore")
import numpy as np

N = 256
P = 128
NE = 8  # elems per core
NT = 4  # output times per core
NCORES = 8
DT = 1e-3
NU = 1e-2
A = DT * NU
NID = 13  # I, 4x aj*I, 4x (c1j*I, c2j*I)

# per-(e,j) output lane: 'A' (PE3+ACT), 'B' (PE2+DVE), 'C' (DVE stt + Pool add)
LANES = {}
for _e in range(NE):
    for _j in range(NT):
        LANES[(_e, _j)] = ["A", "C", "B", "C"][_j] if _e % 2 == 0 else ["C", "A", "A", "C"][_j]
# C-lane second-op engine: all Pool (stt is DVE-only on TRN2)
C_ADD_DVE = set()
# E-evac engine per lap source: 'D' = DVE add, 'A' = PE I@cs + ACT copy
EVAC = {"q": "A", 0: "D", 1: "A", 2: "A", 3: "D", 4: "D", 5: "A", 6: "A", 7: "D"}
# colsum engine per lap source: True = Pool, False = DVE
CS_POOL = {"q": True, 0: True, 1: False, 2: True, 3: False, 4: True, 5: False, 6: True, 7: False}

_compiled = None


def _make_tables(tq):
    # Row-stencil S: S[r, r+-1 mod 256] = 1, S[r,r] = -4 (carries the -4).
    S = np.zeros((N, N), np.float32)
    i = np.arange(N)
    S[i, (i + 1) % N] = 1.0
    S[i, (i - 1) % N] = 1.0
    S[i, i] = -4.0
    st = np.empty((P, 4 * P), np.float16)  # [m*2+h] = S[m-block, h-block].T
    for m in range(2):
        for h in range(2):
            st[:, P * (m * 2 + h) : P * (m * 2 + h + 1)] = S[
                P * m : P * m + P, P * h : P * h + P
            ].T
    ident = np.eye(P, dtype=np.float32)
    ids = np.empty((P, NID * P), np.float16)
    sc = np.zeros(16, np.float32)
    ids[:, 0:P] = ident
    for j in range(NT):
        m = 16 * (4 * tq + j + 1)
        ids[:, (1 + j) * P : (2 + j) * P] = (A * m) * ident
        ids[:, (5 + 2 * j) * P : (6 + 2 * j) * P] = (DT * m) * ident
        ids[:, (6 + 2 * j) * P : (7 + 2 * j) * P] = (
            DT * A * m * (m - 1) / 2.0
        ) * ident
        sc[j] = A * m
    return st, ids, np.tile(sc, (P, 1))


def _build():
    import concourse.bacc as bacc
    import concourse.mybir as mybir
    from concourse.tile import TileContext

    f32 = mybir.dt.float32
    f16 = mybir.dt.float16
    nc = bacc.Bacc("TRN2", target_bir_lowering=False, debug=False)

    # host layouts put partition dim first so grouped DMAs are contiguous
    z_d = nc.dram_tensor("z", [P, NE, 2, N], f16, kind="ExternalInput")
    q_d = nc.dram_tensor("q", [P, 2, N], f16, kind="ExternalInput")
    st_d = nc.dram_tensor("st", [P, 4 * P], f16, kind="ExternalInput")
    id_d = nc.dram_tensor("ids", [P, NID * P], f16, kind="ExternalInput")
    sc_d = nc.dram_tensor("sc", [P, 16], f32, kind="ExternalInput")
    out_d = nc.dram_tensor("out", [P, NE, NT, 2, N], f16, kind="ExternalOutput")

    with TileContext(nc) as tc:
        with (
            tc.tile_pool(name="const", bufs=1) as cpool,
            tc.tile_pool(name="zs", bufs=1) as zpool,
            tc.tile_pool(name="cs", bufs=3) as cspool,
            tc.tile_pool(name="es", bufs=4) as epool,
            tc.tile_pool(name="ct", bufs=4) as ctpool,
            tc.tile_pool(name="outp", bufs=3) as opool,
            tc.tile_pool(name="pse", bufs=2, space="PSUM") as pse,
            tc.tile_pool(name="pso", bufs=6, space="PSUM") as pso,
        ):
            _uid = [0]

            def nm(tag):
                _uid[0] += 1
                return f"{tag}_{_uid[0]}"

            # ---- const loads (grouped) ----
            st_t = cpool.tile([P, 4 * P], f16, tag="st", name=nm("st"))
            nc.sync.dma_start(out=st_t[:, :], in_=st_d.ap()[:, :])
            id_t = cpool.tile([P, NID * P], f16, tag="ids", name=nm("ids"))
            nc.sync.dma_start(out=id_t[:, :], in_=id_d.ap()[:, :])
            sc_t = cpool.tile([P, 16], f32, tag="sc", name=nm("sc"))
            nc.sync.dma_start(out=sc_t[:, :], in_=sc_d.ap()[:, :])
            q_t = cpool.tile([P, 2, N], f16, tag="q", name=nm("q"))
            nc.sync.dma_start(out=q_t[:, :, :], in_=q_d.ap()[:, :, :])
            zA = zpool.tile([P, 4, 2, N], f16, tag="zA", name=nm("zA"))
            nc.sync.dma_start(out=zA[:, :, :, :], in_=z_d.ap()[:, 0:4])
            zB = zpool.tile([P, 4, 2, N], f16, tag="zB", name=nm("zB"))
            nc.sync.dma_start(out=zB[:, :, :, :], in_=z_d.ap()[:, 4:8])

            def zt(e):
                return (zA if e < 4 else zB)[:, e % 4]

            def idb(k):  # identity block k as [128,128] lhsT
                return id_t[:, k * P : (k + 1) * P]

            def stb(m, h):
                return st_t[:, (m * 2 + h) * P : (m * 2 + h + 1) * P]

            def rowpart(ps, src3, cs=None):
                """psum[:, m, :] = S_row @ src (+ I @ cs), per-region groups."""
                for m in range(2):
                    for h in range(2):
                        nc.tensor.matmul(
                            ps[:, m, :],
                            stb(m, h),
                            src3[:, h, :],
                            start=(h == 0),
                            stop=(h == 1 and cs is None),
                        )
                    if cs is not None:
                        nc.tensor.matmul(
                            ps[:, m, :], idb(0), cs[:, m, :], start=False, stop=True
                        )

            def colsum(cs, src3, pool):
                """cs[:,:,c] = src[:,:,c-1] + src[:,:,c+1], periodic (3 ops)."""
                eng = nc.gpsimd if pool else nc.vector
                eng.tensor_add(
                    cs[:, :, 1 : N - 1], src3[:, :, 0 : N - 2], src3[:, :, 2:N]
                )
                eng.tensor_add(
                    cs[:, :, 0:1], src3[:, :, N - 1 : N], src3[:, :, 1:2]
                )
                eng.tensor_add(
                    cs[:, :, N - 1 : N], src3[:, :, N - 2 : N - 1], src3[:, :, 0:1]
                )
            def lap(src3, key, out_tag):
                """E = S_row@src + colsum(src) -> fp16 tile [P,2,N]."""
                cs = cspool.tile([P, 2, N], f16, tag="cs", name=nm("cs"))
                colsum(cs, src3, CS_POOL[key])
                ps = pse.tile([P, 2, N], f32, tag="pse", name=nm("pse"))
                via_act = EVAC[key] == "A"
                lt = epool.tile([P, 2, N], f16, tag=out_tag, name=nm(out_tag))
                if via_act:
                    rowpart(ps, src3, cs=cs)
                    nc.scalar.copy(out=lt[:, :, :], in_=ps[:, :, :])
                else:
                    rowpart(ps, src3)
                    nc.vector.tensor_add(lt[:, :, :], ps[:, :, :], cs[:, :, :])
                return lt

            # ---- forcing prep: LQ, R_j = c1j Q + c2j LQ ----
            lq_t = lap(q_t, "q", "lq")
            r_t = []
            for j in range(NT):
                ps = pse.tile([P, 2, N], f32, tag="pse", name=nm("psr"))
                nc.tensor.matmul(ps[:, :, :], idb(5 + 2 * j), q_t[:, :, :], start=True, stop=False)
                nc.tensor.matmul(ps[:, :, :], idb(6 + 2 * j), lq_t[:, :, :], start=False, stop=True)
                rt = cpool.tile([P, 2, N], f16, tag=f"r{j}", name=nm("r"))
                nc.scalar.copy(out=rt[:, :, :], in_=ps[:, :, :])
                r_t.append(rt)

            # ---- per elem: E = Lap(Z); 4 outputs via lanes ----
            for e in range(NE):
                z3 = zt(e)
                e_t = lap(z3, e, "E")
                o_t = opool.tile([P, NT, 2, N], f16, tag="o", name=nm("o"))
                for j in range(NT):
                    lane = LANES[(e, j)]
                    oj = o_t[:, j]
                    if lane in ("A", "B"):
                        ps = pso.tile([P, 2, N], f32, tag="pso", name=nm("pso"))
                        nc.tensor.matmul(ps[:, :, :], idb(1 + j), e_t[:, :, :], start=True, stop=False)
                        if lane == "A":
                            nc.tensor.matmul(ps[:, :, :], idb(0), z3[:, :, :], start=False, stop=False)
                        nc.tensor.matmul(ps[:, :, :], idb(0), r_t[j][:, :, :], start=False, stop=True)
                        if lane == "A":
                            nc.scalar.copy(out=oj[:, :, :], in_=ps[:, :, :])
                        else:
                            nc.vector.tensor_add(oj[:, :, :], ps[:, :, :], z3[:, :, :])
                    else:  # C: no PSUM
                        import concourse.mybir as mybir

                        t1 = ctpool.tile([P, 2, N], f16, tag="ct", name=nm("ct"))
                        nc.vector.scalar_tensor_tensor(
                            t1[:, :, :],
                            e_t[:, :, :],
                            sc_t[:, j : j + 1],
                            z3[:, :, :],
                            mybir.AluOpType.mult,
                            mybir.AluOpType.add,
                        )
                        nc.gpsimd.tensor_add(
                            oj[:, :, :], t1[:, :, :], r_t[j][:, :, :]
                        )
                nc.sync.dma_start(out=out_d.ap()[:, e], in_=o_t[:, :, :, :])

    nc.compile()
    return nc


def _get_compiled():
    global _compiled
    if _compiled is None:
        _compiled = _build()
    return _compiled


def _run(inputs_full, Q, trace=False):
    from concourse import bass_utils

    nc = _get_compiled()
    z32 = np.asarray(inputs_full, np.float32)
    # [B,256,256] -> per-core [P, NE, 2, N] fp16 (partition-major)
    zh = z32.reshape(16, 2, P, N).astype(np.float16)  # [B, half, p, col]
    qh = np.ascontiguousarray(
        np.asarray(Q, np.float32).reshape(2, P, N).transpose(1, 0, 2)
    ).astype(np.float16)  # [P,2,N]
    in_maps = []
    for c in range(NCORES):
        eh = c % 2
        tq = c // 2
        st, ids, sc = _make_tables(tq)
        zc = zh[eh * NE : (eh + 1) * NE]  # [NE,2,P,N]
        in_maps.append(
            {
                "z": np.ascontiguousarray(zc.transpose(2, 0, 1, 3)),
                "q": qh,
                "st": st,
                "ids": ids,
                "sc": sc,
            }
        )
    kw = dict(trace=True) if trace else {}
    last_err = None
    for attempt in range(3):
        try:
            res = bass_utils.run_bass_kernel_spmd(
                nc, in_maps, core_ids=list(range(NCORES)), **kw
            )
            break
        except Exception as exc:  # rare transient device error; retry
            last_err = exc
            import time

            time.sleep(5)
    else:
        raise last_err
    out = np.empty((16, 16, N, N), dtype=np.float32)
    for c in range(NCORES):
        eh, tq = c % 2, c // 2
        r = np.asarray(res.results[c]["out"], dtype=np.float32)
        # [P, NE, NT, 2, N] -> [NE, NT, 2, P, N] -> [NE, NT, 256, 256]
        r = r.transpose(1, 2, 3, 0, 4).reshape(NE, NT, N, N)
        out[eh * NE : (eh + 1) * NE, tq * NT : (tq + 1) * NT] = r
    return out, res


def kernel(inputs, Q):
    inputs = np.ascontiguousarray(np.asarray(inputs, dtype=np.float32))
    Q = np.ascontiguousarray(np.asarray(Q, dtype=np.float32))
    out, _ = _run(inputs, Q, trace=False)
    return out


# revision 15
# speedup vs baseline: 2.3234x; 1.2897x over previous
"""Binomial-stencil kernel for nn_Dynamics_2748779069592 (TRN2, 8 cores).

Identity: the step is linear, Z_{k+1} = (I + a·L)Z_k + DT·Q with L the 2D
periodic Laplacian and a = DT*NU = 1e-5.  After m steps:
  Z_m = (I+aL)^m Z0 + DT·Σ_{k<m}(I+aL)^k Q
Binomial expansion (|a·λ(L)·m| <= 0.0205):
  Out_j = Z + α_j·L(Z) + R_j,  α_j = a·m_j,  R_j = DT[C(m_j,1)Q + C(m_j,2)a·LQ]
α_j is exactly linear in j (m_j = m_0+16j) and R_j is linear to 2.6e-6, so
per elem only two PSUM passes are needed:
  BASE = Out_0 = Z + α_0·E + R_0       (E = L(Z))
  STEP = 16a·E + ΔR                    (ΔR = R_1 - R_0, linearized)
  Out_{j+1} = Out_j + STEP             (fp16 DVE adds, 2x mode)
E = S_row@Z + colsum(Z): row part as 4 [128,128]x[128,256] PE matmuls
(banded circulant carrying the -4 diag) + I@colsum accumulated per region;
col part as one big shifted 3D DVE add (2x) + 2 Pool edge ops.
All SBUF data fp16 (PSUM f32); outputs DMA'd as fp16, host upcasts.

Sharding: 8 cores as 2x4 grid — core c owns 8 batch elems (half c%2) and 4
output times (quarter c//2). No cross-core communication.
"""
import sys

sys.path.insert(0, "/opt/trn_rl_repo")
import warnings

warnings.filterwarnings("ignore")
import numpy as np

N = 256
P = 128
NE = 8  # elems per core
NT = 4  # output times per core
NCORES = 8
DT = 1e-3
NU = 1e-2
A = DT * NU
NID = 7  # I, a0*I, 16a*I, c1_0*I, c2_0*I, dc1*I, dc2*I

# E-evac engine per lap source: 'A' = PE I@cs + ACT copy, 'D' = DVE add
EVAC = {"q": "A", 0: "A", 1: "A", 2: "A", 3: "D", 4: "A", 5: "A", 6: "A", 7: "D"}

_compiled = None


def _make_tables(tq):
    # Row-stencil S: S[r, r+-1 mod 256] = 1, S[r,r] = -4 (carries the -4).
    S = np.zeros((N, N), np.float32)
    i = np.arange(N)
    S[i, (i + 1) % N] = 1.0
    S[i, (i - 1) % N] = 1.0
    S[i, i] = -4.0
    st = np.empty((P, 4 * P), np.float16)  # [m*2+h] = S[m-block, h-block].T
    for m in range(2):
        for h in range(2):
            st[:, P * (m * 2 + h) : P * (m * 2 + h + 1)] = S[
                P * m : P * m + P, P * h : P * h + P
            ].T
    ident = np.eye(P, dtype=np.float32)
    m0 = 16 * (4 * tq + 1)
    m1 = m0 + 16

    def c2(m):
        return m * (m - 1) / 2.0

    ids = np.empty((P, NID * P), np.float16)
    for k, v in enumerate(
        [
            1.0,
            A * m0,
            16.0 * A,
            DT * m0,
            DT * A * c2(m0),
            DT * (m1 - m0),
            DT * A * (c2(m1) - c2(m0)),
        ]
    ):
        ids[:, k * P : (k + 1) * P] = v * ident
    return st, ids


def _build():
    import concourse.bacc as bacc
    import concourse.mybir as mybir
    from concourse.tile import TileContext

    f32 = mybir.dt.float32
    f16 = mybir.dt.float16
    nc = bacc.Bacc("TRN2", target_bir_lowering=False, debug=False)

    # host layouts put partition dim first so grouped DMAs are contiguous
    z_d = nc.dram_tensor("z", [P, NE, 2, N], f16, kind="ExternalInput")
    q_d = nc.dram_tensor("q", [P, 2, N], f16, kind="ExternalInput")
    st_d = nc.dram_tensor("st", [P, 4 * P], f16, kind="ExternalInput")
    id_d = nc.dram_tensor("ids", [P, NID * P], f16, kind="ExternalInput")
    out_d = nc.dram_tensor("out", [P, NE, NT, 2, N], f16, kind="ExternalOutput")

    with TileContext(nc) as tc:
        with (
            tc.tile_pool(name="const", bufs=1) as cpool,
            tc.tile_pool(name="zs", bufs=1) as zpool,
            tc.tile_pool(name="cs", bufs=3) as cspool,
            tc.tile_pool(name="es", bufs=4) as epool,
            tc.tile_pool(name="stp", bufs=4) as stpool,
            tc.tile_pool(name="outp", bufs=3) as opool,
            tc.tile_pool(name="pse", bufs=3, space="PSUM") as pse,
            tc.tile_pool(name="pso", bufs=5, space="PSUM") as pso,
        ):
            _uid = [0]

            def nm(tag):
                _uid[0] += 1
                return f"{tag}_{_uid[0]}"

            # ---- const loads: critical path first (st, q), then ids, z ----
            st_t = cpool.tile([P, 4 * P], f16, tag="st", name=nm("st"))
            nc.sync.dma_start(out=st_t[:, :], in_=st_d.ap()[:, :])
            q_t = cpool.tile([P, 2, N], f16, tag="q", name=nm("q"))
            nc.sync.dma_start(out=q_t[:, :, :], in_=q_d.ap()[:, :, :])
            id_t = cpool.tile([P, NID * P], f16, tag="ids", name=nm("ids"))
            nc.sync.dma_start(out=id_t[:, :], in_=id_d.ap()[:, :])
            zA = zpool.tile([P, 4, 2, N], f16, tag="zA", name=nm("zA"))
            nc.sync.dma_start(out=zA[:, :, :, :], in_=z_d.ap()[:, 0:4])
            zB = zpool.tile([P, 4, 2, N], f16, tag="zB", name=nm("zB"))
            nc.sync.dma_start(out=zB[:, :, :, :], in_=z_d.ap()[:, 4:8])

            def zt(e):
                return (zA if e < 4 else zB)[:, e % 4]

            def idb(k):  # identity block k as [128,128] lhsT
                return id_t[:, k * P : (k + 1) * P]

            def stb(m, h):
                return st_t[:, (m * 2 + h) * P : (m * 2 + h + 1) * P]

            def rowpart(ps, src3, cs=None):
                """psum[:, m, :] = S_row @ src (+ I @ cs), per-region groups."""
                for m in range(2):
                    for h in range(2):
                        nc.tensor.matmul(
                            ps[:, m, :],
                            stb(m, h),
                            src3[:, h, :],
                            start=(h == 0),
                            stop=(h == 1 and cs is None),
                        )
                    if cs is not None:
                        nc.tensor.matmul(
                            ps[:, m, :], idb(0), cs[:, m, :], start=False, stop=True
                        )

            def colsum(cs, src3):
                """cs[:,:,c] = src[:,:,c-1] + src[:,:,c+1], periodic.
                Big middle op on DVE (fp16 2x), 2 edge col ops on Pool."""
                nc.vector.tensor_add(
                    cs[:, :, 1 : N - 1], src3[:, :, 0 : N - 2], src3[:, :, 2:N]
                )
                nc.gpsimd.tensor_add(
                    cs[:, :, 0:1], src3[:, :, N - 1 : N], src3[:, :, 1:2]
                )
                nc.gpsimd.tensor_add(
                    cs[:, :, N - 1 : N], src3[:, :, N - 2 : N - 1], src3[:, :, 0:1]
                )

            def lap(src3, key, out_tag):
                """E = S_row@src + colsum(src) -> fp16 tile [P,2,N]."""
                cs = cspool.tile([P, 2, N], f16, tag="cs", name=nm("cs"))
                colsum(cs, src3)
                ps = pse.tile([P, 2, N], f32, tag="pse", name=nm("pse"))
                lt = epool.tile([P, 2, N], f16, tag=out_tag, name=nm(out_tag))
                if EVAC[key] == "A":
                    rowpart(ps, src3, cs=cs)
                    nc.scalar.copy(out=lt[:, :, :], in_=ps[:, :, :])
                else:
                    rowpart(ps, src3)
                    nc.vector.tensor_add(lt[:, :, :], ps[:, :, :], cs[:, :, :])
                return lt

            # ---- forcing prep: LQ, R0 = c1_0 Q + c2_0 LQ, dR = dc1 Q + dc2 LQ
            lq_t = lap(q_t, "q", "lq")
            r0_t = cpool.tile([P, 2, N], f16, tag="r0", name=nm("r0"))
            dr_t = cpool.tile([P, 2, N], f16, tag="dr", name=nm("dr"))
            for rt, ka, kb in ((r0_t, 3, 4), (dr_t, 5, 6)):
                ps = pse.tile([P, 2, N], f32, tag="pse", name=nm("psr"))
                nc.tensor.matmul(ps[:, :, :], idb(ka), q_t[:, :, :], start=True, stop=False)
                nc.tensor.matmul(ps[:, :, :], idb(kb), lq_t[:, :, :], start=False, stop=True)
                nc.scalar.copy(out=rt[:, :, :], in_=ps[:, :, :])

            # ---- per elem: E, BASE (=Out_0), STEP, then 3 chained adds ----
            for e in range(NE):
                z3 = zt(e)
                e_t = lap(z3, e, "E")
                o_t = opool.tile([P, NT, 2, N], f16, tag="o", name=nm("o"))
                # BASE -> o_t[:, 0]
                ps = pso.tile([P, 2, N], f32, tag="pso", name=nm("psb"))
                nc.tensor.matmul(ps[:, :, :], idb(1), e_t[:, :, :], start=True, stop=False)
                nc.tensor.matmul(ps[:, :, :], idb(0), z3[:, :, :], start=False, stop=False)
                nc.tensor.matmul(ps[:, :, :], idb(0), r0_t[:, :, :], start=False, stop=True)
                nc.scalar.copy(out=o_t[:, 0], in_=ps[:, :, :])
                # STEP
                ps2 = pso.tile([P, 2, N], f32, tag="pso", name=nm("pss"))
                nc.tensor.matmul(ps2[:, :, :], idb(2), e_t[:, :, :], start=True, stop=False)
                nc.tensor.matmul(ps2[:, :, :], idb(0), dr_t[:, :, :], start=False, stop=True)
                stp = stpool.tile([P, 2, N], f16, tag="stp", name=nm("stp"))
                nc.scalar.copy(out=stp[:, :, :], in_=ps2[:, :, :])
                # chain: Out_{j+1} = Out_j + STEP (DVE fp16 2x)
                for j in range(1, NT):
                    nc.vector.tensor_add(o_t[:, j], o_t[:, j - 1], stp[:, :, :])
                nc.sync.dma_start(out=out_d.ap()[:, e], in_=o_t[:, :, :, :])

    nc.compile()
    return nc


def _get_compiled():
    global _compiled
    if _compiled is None:
        _compiled = _build()
    return _compiled


def _run(inputs_full, Q, trace=False):
    from concourse import bass_utils

    nc = _get_compiled()
    z32 = np.asarray(inputs_full, np.float32)
    # [B,256,256] -> per-core [P, NE, 2, N] fp16 (partition-major)
    zh = z32.reshape(16, 2, P, N).astype(np.float16)  # [B, half, p, col]
    qh = np.ascontiguousarray(
        np.asarray(Q, np.float32).reshape(2, P, N).transpose(1, 0, 2)
    ).astype(np.float16)  # [P,2,N]
    in_maps = []
    for c in range(NCORES):
        eh = c % 2
        tq = c // 2
        st, ids = _make_tables(tq)
        zc = zh[eh * NE : (eh + 1) * NE]  # [NE,2,P,N]
        in_maps.append(
            {
                "z": np.ascontiguousarray(zc.transpose(2, 0, 1, 3)),
                "q": qh,
                "st": st,
                "ids": ids,
            }
        )
    kw = dict(trace=True) if trace else {}
    last_err = None
    for attempt in range(3):
        try:
            res = bass_utils.run_bass_kernel_spmd(
                nc, in_maps, core_ids=list(range(NCORES)), **kw
            )
            break
        except Exception as exc:  # rare transient device error; retry
            last_err = exc
            import time

            time.sleep(5)
    else:
        raise last_err
    out = np.empty((16, 16, N, N), dtype=np.float32)
    for c in range(NCORES):
        eh, tq = c % 2, c // 2
        r = np.asarray(res.results[c]["out"], dtype=np.float32)
        # [P, NE, NT, 2, N] -> [NE, NT, 2, P, N] -> [NE, NT, 256, 256]
        r = r.transpose(1, 2, 3, 0, 4).reshape(NE, NT, N, N)
        out[eh * NE : (eh + 1) * NE, tq * NT : (tq + 1) * NT] = r
    return out, res


def kernel(inputs, Q):
    inputs = np.ascontiguousarray(np.asarray(inputs, dtype=np.float32))
    Q = np.ascontiguousarray(np.asarray(Q, dtype=np.float32))
    out, _ = _run(inputs, Q, trace=False)
    return out
